# revision 35
# baseline (speedup 1.0000x reference)
"""GSPN Trainium2 kernel: batch x channel-half sharding over 8 cores.

Core c = 2*b + half.  Each core computes batch b; channels are split in
half through the middle of the network.  v2 redesign vs baseline:
  - contiguous moving operands for gate / outconv matmuls (w-major xp,
    h-major out_m copy) -- the baseline's strided rhs ran the PE ~3x slow
  - gate matmuls alternate PE row groups (rows 0:48 vs 64:112 carry
    duplicated weights/xp) so LDWEIGHTS pulls ahead of in-flight MMs
  - gates/scan/merge in two 2-round groups, bf16 elementwise (DVE 2x),
    scan state padded to 36 cols so most ops keep 4B alignment
  - xdown AllReduce split in two halves; keepalive matmuls keep the PE
    HAM clock warm across collective waits and the DVE scan
  - outconv / outproj ReduceScatters chunked + bf16, overlapped with
    the producing matmuls
"""

import numpy as np
import ml_dtypes

import concourse.bass as bass
import concourse.mybir as mybir
import concourse.tile as tile
from concourse import bacc, bass_utils

F32 = mybir.dt.float32
BF16 = mybir.dt.bfloat16
AF = mybir.ActivationFunctionType
OP = mybir.AluOpType

D = 768
DH = 384          # channels per half
P = 1024          # pixels
HW = 32
CT = 3            # channel tiles per half
SCT = 12          # scan channel tiles (4 dirs x 3)
NR = 4            # rounds (8-col scan chunks)
CW = 8            # columns per chunk
RG = 2            # rounds per group
HP = 36           # padded h dim for scan state (zeros at 0,1 and 34,35)
EPS = 1e-5

REPLICA_PAIRS = [[0, 1], [2, 3], [4, 5], [6, 7]]


def build_program():
    nc = bacc.Bacc("TRN2", target_bir_lowering=False, debug=False,
                   enable_asserts=True, num_devices=8)

    def din(name, shape, dt):
        return nc.dram_tensor(name, shape, dt, kind="ExternalInput").ap()

    hs = din("hs", [P, D], F32)
    w_in = din("w_in", [128, 6 * DH], BF16)       # in_proj lhsT, (ktile, M) packed
    b_in = din("b_in", [1, DH], BF16)
    dw7 = din("dw7", [CT, 128, 49 * 128], BF16)   # diag taps per ctile
    b7 = din("b7", [1, DH], BF16)
    w_xd = din("w_xd", [128, CT * 48], BF16)
    w_g = din("w_g", [128, 72 * 128], BF16)       # rows 0:48 (and dup 64:112)
    w_oc = din("w_oc", [128, CT * D], BF16)
    dw3 = din("dw3", [128, CT * 9 * 128], BF16)
    w_op = din("w_op", [128, CT * D], BF16)
    ident = din("ident", [128, 128], BF16)
    y_out = nc.dram_tensor("y", [P // 2, D], F32, kind="ExternalOutput").ap()

    with tile.TileContext(nc) as tc:
        with tc.tile_pool(name="wp", bufs=1) as wp, \
             tc.tile_pool(name="mid", bufs=1) as mid, \
             tc.tile_pool(name="wps", bufs=1, space="PSUM") as wps, \
             tc.tile_pool(name="dram", bufs=1, space="DRAM") as dramp:

            # ---- persistent weights ----
            w_in_sb = wp.tile([128, 6 * DH], BF16, tag="w_in")
            nc.sync.dma_start(w_in_sb[:], w_in[:])
            b_in_sb = wp.tile([1, DH], BF16, tag="b_in")
            nc.sync.dma_start(b_in_sb[:], b_in[:])
            b7_sb = wp.tile([1, DH], BF16, tag="b7")
            nc.sync.dma_start(b7_sb[:], b7[:])
            w_xd_sb = wp.tile([128, CT * 48], BF16, tag="w_xd")
            nc.sync.dma_start(w_xd_sb[:], w_xd[:])
            # w_g / tail weights are DMA'd later so hs tiles win the queue
            w_g_sb = wp.tile([128, 72 * 128], BF16, tag="w_g")
            w_oc_sb = wp.tile([128, CT * D], BF16, tag="w_oc")
            dw3_sb = wp.tile([128, CT * 9 * 128], BF16, tag="dw3")
            w_op_sb = wp.tile([128, CT * D], BF16, tag="w_op")
            ident_sb = wp.tile([128, 128], BF16, tag="ident")
            nc.sync.dma_start(ident_sb[:], ident[:])
            ones_sb = wp.tile([1, 512], BF16, tag="ones")
            nc.vector.memset(ones_sb[:], 1.0)
            cst = wp.tile([128, 3], F32, tag="cst")
            nc.gpsimd.memset(cst[:, 0:1], EPS)
            nc.gpsimd.memset(cst[:, 1:2], -0.5)
            nc.gpsimd.memset(cst[:, 2:3], -1.0)

            # keepalive scratch: tiny matmuls keep the PE HAM clock warm
            warm_ps = wps.tile([64, 64], F32, tag="warm")

            def poke(n, lhs=None, rhs=None):
                lh = lhs if lhs is not None else w_in_sb[0:48, 0:64]
                rh = rhs if rhs is not None else w_in_sb[0:48, 64:128]
                for _ in range(n):
                    nc.tensor.matmul(warm_ps[:], lh, rh, start=True, stop=True)

            # tiny pair AllReduce with a data dependency: fires when `dep`
            # is written, keeping the ncfw collective path warm (a cold
            # collective costs ~15-25us extra)
            ncp = [0]

            def cpoke(dep):
                ncp[0] += 1
                cbi = dramp.tile([1, 16], BF16, tag=f"cwb{ncp[0]}", name=f"cwb{ncp[0]}")
                cbo = dramp.tile([1, 16], BF16, tag=f"cwo{ncp[0]}", name=f"cwo{ncp[0]}")
                nc.sync.dma_start(cbi[:], dep)
                nc.gpsimd.collective_compute(
                    "AllReduce", OP.add, replica_groups=REPLICA_PAIRS,
                    ins=[cbi.opt()], outs=[cbo.opt()])

            # ---- mid-lifetime tensors ----
            x2h = mid.tile([128, CT * P], BF16, tag="x2h")   # (ct, h, w)
            x2w = mid.tile([128, CT * P], BF16, tag="x2w")   # (ct, w, h)
            xpb = mid.tile([128, P], BF16, tag="xpb")        # w-major, rows 0:48 + 64:112
            out_m = mid.tile([128, CT * P], BF16, tag="out_m")   # (ct, w, h)

            x2h4 = x2h[:].rearrange("p (c h w) -> p c h w", c=CT, h=HW, w=HW)
            x2w4 = x2w[:].rearrange("p (c w h) -> p c w h", c=CT, w=HW, h=HW)
            # scan-sliced 5D views: (ct, round, col-in-round, stencil)
            x2w5 = x2w[:].rearrange("p (c r w h) -> p c r w h", c=CT, r=NR, w=CW, h=HW)
            x2h5 = x2h[:].rearrange("p (c r w h) -> p c r w h", c=CT, r=NR, w=CW, h=HW)
            out_m4 = out_m[:].rearrange("p (c w h) -> p c w h", c=CT, w=HW, h=HW)
            om6 = out_m[:].rearrange("p (c g r w h) -> p c g r w h",
                                     c=CT, g=2, r=RG, w=CW, h=HW)

            # S8 padded image; memset early, off the critical path
            ocp = mid.tile([128, CT * 36 * 36], BF16, tag="ocp")
            nc.vector.memset(ocp[:], 0.0)
            ocp4 = ocp[:].rearrange("p (c a b) -> p c a b", c=CT, a=36, b=36)

            poke(100)   # pre-warm the PE while LN stats run

            # warm up the collective path early: tiny AllReduce absorbs the
            # ~20us ncfw first-collective cost while LN/conv7 run
            cwarm = wp.tile([1, 16], F32, tag="cwarm")
            nc.vector.memset(cwarm[:], 0.0)
            cw_bi = dramp.tile([1, 16], F32, tag="cw_bi")
            cw_bo = dramp.tile([1, 16], F32, tag="cw_bo")
            nc.sync.dma_start(cw_bi[:], cwarm[:])
            nc.gpsimd.collective_compute(
                "AllReduce", OP.add, replica_groups=REPLICA_PAIRS,
                ins=[cw_bi.opt()], outs=[cw_bo.opt()])

            # ================= S0: LN (batched stats) + PE transpose =================
            with tc.tile_pool(name="early", bufs=1) as ep, \
                 tc.tile_pool(name="lnp", bufs=2) as lnp, \
                 tc.tile_pool(name="pse", bufs=2, space="PSUM") as pse:

                xT = ep.tile([128, 6 * P], BF16, tag="xT")       # (ktile, pix)
                xh_all = ep.tile([128, 8 * D], F32, tag="xh_all")
                xh3 = xh_all[:].rearrange("p (i d) -> p i d", i=8)
                xh4 = xh_all[:].rearrange("p (i c d) -> p i c d", i=8, c=2)
                bst = ep.tile([128, 8 * 12], F32, tag="bst")
                bst3 = bst[:].rearrange("p (i s) -> p i s", i=8)       # [p, 8, 12]
                bst4 = bst[:].rearrange("p (i c s) -> p i c s", i=8, c=2)
                st = ep.tile([128, 16], F32, tag="st")                 # (i, [mean, var])
                stv = st[:].rearrange("p (i s) -> p i s", s=2)
                rs8 = ep.tile([128, 8], F32, tag="rs8")
                rs8v = rs8[:].rearrange("p (i o) -> p i o", o=1)

                for i in range(8):
                    nc.sync.dma_start(xh3[:, i, :], hs[i * 128:(i + 1) * 128, :])
                # heavy weight loads queue behind the hs tiles
                nc.sync.dma_start(w_g_sb[:], w_g[:])
                for i in range(8):
                    nc.vector.bn_stats(bst4[:, i, 0], xh4[:, i, 0])
                    nc.vector.bn_stats(bst4[:, i, 1], xh4[:, i, 1])
                    nc.vector.bn_aggr(st[:, 2 * i:2 * i + 2], bst3[:, i, :])
                # rs = exp(-0.5*ln(var+eps)) for all 8 tiles in two scalar ops
                nc.scalar.activation(rs8v[:], stv[:, :, 1:2], AF.Ln, bias=cst[:, 0:1])
                nc.scalar.activation(rs8[:], rs8[:], AF.Exp, scale=cst[:, 1:2])

                x1p = ep.tile([128, CT * 1600], BF16, tag="x1p")  # 40x40 padded
                nc.vector.memset(x1p[:], 0.0)

                xT2 = xT[:].rearrange("p (i k t) -> p i k t", i=8, k=6)
                for i in range(8):
                    xhb = lnp.tile([128, D], BF16, tag="xhb")
                    nc.vector.tensor_scalar(xhb[:], xh3[:, i, :], st[:, 2 * i:2 * i + 1],
                                            rs8[:, i:i + 1],
                                            op0=OP.subtract, op1=OP.mult)
                    for g in range(2):   # ktile groups 0-3, 4-5
                        kn = 4 if g == 0 else 2
                        pt = pse.tile([128, 512], BF16, tag="tp")
                        for kk in range(kn):
                            k = g * 4 + kk
                            nc.tensor.transpose(pt[:, kk * 128:(kk + 1) * 128],
                                                xhb[:, k * 128:(k + 1) * 128],
                                                ident_sb[:])
                        # (i, k, t) layout -> contiguous eviction
                        dst = xT2[:, i, g * 4:g * 4 + kn, :]
                        src = pt[:, 0:kn * 128].rearrange("p (k t) -> p k t", k=kn)
                        nc.scalar.activation(dst, src, AF.Copy)

                # ================= S1: in_proj (+LN bias) =================
                x1p4 = x1p[:].rearrange("p (c a b) -> p c a b", c=CT, a=40, b=40)
                for m in range(CT):
                    for nh in range(2):
                        ps = pse.tile([128, 512], F32, tag="e")
                        for k in range(6):
                            nc.tensor.matmul(ps[:],
                                             w_in_sb[:, k * DH + m * 128:k * DH + (m + 1) * 128],
                                             xT2[:, nh * 4:(nh + 1) * 4, k, :],
                                             start=(k == 0), stop=False)
                        nc.tensor.matmul(ps[:], b_in_sb[:, m * 128:(m + 1) * 128],
                                         ones_sb[:, 0:512], start=False, stop=True)
                        dst = x1p4[:, m, nh * 16 + 4:nh * 16 + 20, 4:36]
                        src = ps[:].rearrange("p (a b) -> p a b", a=16)
                        nc.scalar.activation(dst, src, AF.Copy)

                # ====== S2: conv7 via diagonal taps (+ xdown folded per ctile) ======
                xdp = []
                with tc.tile_pool(name="dw7p", bufs=2) as dw7p, \
                     tc.tile_pool(name="psx", bufs=2, space="PSUM") as psx:
                    for nh in range(2):
                        xdp.append(psx.tile([48, 512], F32, tag="xd", name=f"xdp{nh}"))
                    for ct in range(CT):
                        dwt = dw7p.tile([128, 49 * 128], BF16, tag="dwt")
                        nc.sync.dma_start(dwt[:], dw7[ct, :, :])
                        for nh in range(2):
                            ps = pse.tile([128, 512], F32, tag="e")
                            for tap in range(49):
                                dy, dx = tap // 7, tap % 7
                                rhs = x1p4[:, ct, nh * 16 + 1 + dy:nh * 16 + 17 + dy,
                                           1 + dx:33 + dx]
                                nc.tensor.matmul(ps[:], dwt[:, tap * 128:(tap + 1) * 128],
                                                 rhs, start=(tap == 0), stop=False)
                            nc.tensor.matmul(ps[:], b7_sb[:, ct * 128:(ct + 1) * 128],
                                             ones_sb[:, 0:512], start=False, stop=True)
                            dst = x2h4[:, ct, nh * 16:nh * 16 + 16, :]
                            nc.scalar.activation(dst, ps[:].rearrange("p (a b) -> p a b", a=16),
                                                 AF.Copy)
                            # xdown partial accumulates as conv7 output lands
                            nc.tensor.matmul(xdp[nh][:], w_xd_sb[:, ct * 48:(ct + 1) * 48],
                                             x2h[:, ct * P + nh * 512:ct * P + (nh + 1) * 512],
                                             start=(ct == 0), stop=(ct == CT - 1))
                        # transposed copy (w-major)
                        nc.vector.tensor_copy(x2w4[:, ct], x2h4[:, ct].transpose([0, 2, 1]))

                    # ============ S3: split AllReduce of the xdown partial ============
                    xp_wm = ep.tile([48, P], F32, tag="xp_wm")   # (w, h) layout
                    xp_wm3 = xp_wm[:].rearrange("p (w h) -> p w h", w=HW)
                    for nh in range(2):
                        # psum holds (h16, w32); write transposed into (w, h)
                        nc.vector.tensor_copy(
                            xp_wm3[:, :, nh * 16:(nh + 1) * 16],
                            xdp[nh][:].rearrange("p (h w) -> p h w", h=16).transpose([0, 2, 1]))

                    xpf = mid.tile([128, P], F32, tag="xpf")
                    xp_bi = dramp.tile([48, P], F32, tag="xp_bi")
                    xp_bo = dramp.tile([48, P], F32, tag="xp_bo")
                    nc.sync.dma_start(xp_bi[:], xp_wm[:])
                    nc.gpsimd.collective_compute(
                        "AllReduce", OP.add, replica_groups=REPLICA_PAIRS,
                        ins=[xp_bi.opt()], outs=[xp_bo.opt()])
                    nc.sync.dma_start(xpf[0:48, :], xp_bo[:])
                    nc.gpsimd.dma_start(xpf[64:112, :], xp_bo[:])
                    nc.vector.tensor_copy(xpb[0:48, :], xpf[0:48, :])
                    nc.vector.tensor_copy(xpb[64:112, :], xpf[64:112, :])
                    poke(120)   # keep PE warm across the collective wait

            # tail weights arrive during the mid phase
            nc.sync.dma_start(w_oc_sb[:], w_oc[:])
            nc.sync.dma_start(dw3_sb[:], dw3[:])
            nc.sync.dma_start(w_op_sb[:], w_op[:])

            # ====== S4-S6: gates + scan + merge, two 2-round groups ======
            with tc.tile_pool(name="gp", bufs=1) as gp, \
                 tc.tile_pool(name="psg", bufs=6, space="PSUM") as psg:
                glt = gp.tile([128, SCT * RG * CW * HW], BF16, tag="glt")
                gmt = gp.tile([128, SCT * RG * CW * HW], BF16, tag="gmt")
                grt = gp.tile([128, SCT * RG * CW * HW], BF16, tag="grt")
                rrt = gp.tile([128, SCT * RG * CW * HW], BF16, tag="rrt")
                lxt = gp.tile([128, SCT * RG * CW * HW], BF16, tag="lxt")
                uut = gp.tile([128, SCT * RG * CW * HW], BF16, tag="uut")
                sc = gp.tile([128, SCT * RG * CW * HP], BF16, tag="sc")
                ta = gp.tile([128, SCT * RG * HW], BF16, tag="ta")
                tb = gp.tile([128, SCT * RG * HW], BF16, tag="tb")

                def v7(t):
                    return t[:].rearrange("p (c r w h) -> p c r w h", c=SCT, r=RG, w=CW)
                gl5, gm5, gr5, rr5 = v7(glt), v7(gmt), v7(grt), v7(rrt)
                lx5 = lxt[:].rearrange("p (c r w h) -> p c r w h", c=SCT, r=RG, w=CW)
                uu5 = uut[:].rearrange("p (c r w h) -> p c r w h", c=SCT, r=RG, w=CW)
                sc5 = sc[:].rearrange("p (c r w h) -> p c r w h", c=SCT, r=RG, w=CW)
                ta4 = ta[:].rearrange("p (c r h) -> p c r h", c=SCT, r=RG)
                tb4 = tb[:].rearrange("p (c r h) -> p c r h", c=SCT, r=RG)

                def xs_ap(c, g):
                    d, c3 = c // 3, c % 3
                    v = (x2w5 if d in (0, 2) else x2h5)[:, c3]   # [p, r4, w8, h32]
                    if d >= 2:
                        v = v[:, ::-1, ::-1, :]                  # flip scan dim
                    return v[:, 2 * g:2 * g + 2]                 # [p, 2, 8, 32]

                # zero scan-state halo columns once (never written by the scan)
                nc.vector.memset(sc5[:, :, :, :, 0:2], 0.0)
                nc.vector.memset(sc5[:, :, :, :, HP - 2:HP], 0.0)

                for g in range(2):
                    # --- gates: alternate row groups 0:48 / 64:112 for LDW overlap ---
                    for c in range(SCT):
                        for ti in range(6):
                            m = ti * 12 + c
                            r0 = 0 if (m % 2 == 0) else 64
                            ps = psg.tile([128, 512], F32, tag="g")
                            ps4 = ps[:].rearrange("p (r w h) -> p r w h", r=RG, w=CW)
                            nc.tensor.matmul(ps[:],
                                             w_g_sb[r0:r0 + 48, m * 128:(m + 1) * 128],
                                             xpb[r0:r0 + 48, g * 512:(g + 1) * 512],
                                             start=True, stop=True)
                            if ti == 0:
                                nc.scalar.activation(gl5[:, c], ps4, AF.Sigmoid)
                            elif ti == 1:
                                nc.scalar.activation(gm5[:, c], ps4, AF.Sigmoid)
                            elif ti == 2:
                                nc.scalar.activation(gr5[:, c], ps4, AF.Sigmoid)
                            elif ti == 3:
                                nc.vector.tensor_mul(lx5[:, c], ps4, xs_ap(c, g))
                            elif ti == 4:
                                nc.scalar.activation(uu5[:, c], ps4, AF.Copy)
                            else:
                                # xd term accumulated straight into out_m
                                j, dd = c % 3, c // 3
                                acc = om6[:, j, g]
                                if dd == 0:
                                    nc.vector.tensor_mul(acc, ps4, xs_ap(c, g))
                                else:
                                    tmj = gp.tile([128, RG * CW * HW], BF16, tag="tmj")
                                    tm4 = tmj[:].rearrange("p (r w h) -> p r w h",
                                                           r=RG, w=CW)
                                    nc.vector.tensor_mul(tm4, ps4, xs_ap(c, g))
                                    nc.vector.tensor_add(acc, acc, tm4)

                        # s = gl+gm+gr (boundary-fixed) and rr = 1/s, in
                        # 4-ctile chunks pipelined under the gate matmuls
                        if c % 4 == 3:
                            lo = (c - 3) * RG * CW * HW
                            hi = (c + 1) * RG * CW * HW
                            cs = slice(c - 3, c + 1)
                            nc.vector.tensor_add(rrt[:, lo:hi], glt[:, lo:hi],
                                                 gmt[:, lo:hi])
                            nc.vector.tensor_add(rrt[:, lo:hi], rrt[:, lo:hi],
                                                 grt[:, lo:hi])
                            nc.vector.tensor_sub(rr5[:, cs, :, :, 0:1],
                                                 rr5[:, cs, :, :, 0:1],
                                                 gl5[:, cs, :, :, 0:1])
                            nc.vector.tensor_sub(rr5[:, cs, :, :, HW - 1:HW],
                                                 rr5[:, cs, :, :, HW - 1:HW],
                                                 gr5[:, cs, :, :, HW - 1:HW])
                            nc.scalar.activation(rrt[:, lo:hi], rrt[:, lo:hi], AF.Ln)
                            nc.scalar.activation(rrt[:, lo:hi], rrt[:, lo:hi],
                                                 AF.Exp, scale=cst[:, 2:3])

                    # --- scan (both rounds of the group batched) ---
                    nc.vector.tensor_copy(sc5[:, :, :, 0, 2:34], lx5[:, :, :, 0, :])
                    for t in range(1, CW):
                        hp = sc5[:, :, :, t - 1, :]
                        nc.vector.tensor_mul(ta4, gl5[:, :, :, t, :], hp[:, :, :, 1:33])
                        nc.vector.tensor_mul(tb4, gm5[:, :, :, t, :], hp[:, :, :, 2:34])
                        nc.vector.tensor_add(ta4, ta4, tb4)
                        poke(1, lhs=tb[:, 0:64], rhs=w_g_sb[:, 0:64])
                        nc.vector.tensor_mul(tb4, gr5[:, :, :, t, :], hp[:, :, :, 3:35])
                        nc.vector.tensor_add(ta4, ta4, tb4)
                        nc.vector.tensor_mul(ta4, ta4, rr5[:, :, :, t, :])
                        nc.vector.tensor_add(sc5[:, :, :, t, 2:34], ta4, lx5[:, :, :, t, :])
                        poke(1, lhs=ta[:, 0:64], rhs=w_g_sb[:, 0:64])
                        poke(1, lhs=tb[:, 64:128], rhs=w_g_sb[:, 0:64])
                        if g == 1 and t == 5:
                            cpoke(ta[0:1, 0:16])

                    # --- merge: out_m += sum_d sc*U' ---
                    for j in range(CT):
                        acc = om6[:, j, g]
                        for dd in range(4):
                            c = 3 * dd + j
                            tmg = gp.tile([128, RG * CW * HW], BF16, tag="tmg")
                            tm4 = tmg[:].rearrange("p (r w h) -> p r w h", r=RG, w=CW)
                            nc.vector.tensor_mul(tm4, sc5[:, c, :, :, 2:34], uu5[:, c])
                            nc.vector.tensor_add(acc, acc, tm4)
                            poke(1, lhs=tmg[:, 0:64], rhs=w_g_sb[:, 0:64])

            # ================= S7-S9: tail =================
            with tc.tile_pool(name="tail", bufs=1) as tp2, \
                 tc.tile_pool(name="tevict", bufs=4) as tev, \
                 tc.tile_pool(name="pst", bufs=2, space="PSUM") as pst:

                # h-major copy of out_m for contiguous outconv rhs
                out_mT = tp2.tile([128, CT * P], BF16, tag="out_mT")
                out_mT4 = out_mT[:].rearrange("p (c h w) -> p c h w", c=CT, h=HW, w=HW)
                for ct in range(CT):
                    nc.vector.tensor_copy(out_mT4[:, ct], out_m4[:, ct].transpose([0, 2, 1]))

                # --- S7: outconv partial + one bf16 ReduceScatter ---
                oc_bi = dramp.tile([6, 128, P], BF16, tag="oc_bi")
                oc_bo = dramp.tile([CT, 128, P], BF16, tag="oc_bo")
                for m in range(6):
                    ocsb = tev.tile([128, P], BF16, tag="ocsb")
                    for nh in range(2):
                        ps = pst.tile([128, 512], F32, tag="e")
                        for k in range(CT):
                            rhs = out_mT4[:, k, nh * 16:nh * 16 + 16, :]
                            nc.tensor.matmul(ps[:],
                                             w_oc_sb[:, k * D + m * 128:k * D + (m + 1) * 128],
                                             rhs, start=(k == 0), stop=(k == CT - 1))
                        nc.scalar.activation(ocsb[:, nh * 512:(nh + 1) * 512],
                                             ps[:], AF.Copy)
                    nc.sync.dma_start(oc_bi[m, :, :], ocsb[:])
                nc.gpsimd.collective_compute(
                    "ReduceScatter", OP.add, replica_groups=REPLICA_PAIRS,
                    ins=[oc_bi.opt()], outs=[oc_bo.opt()])
                poke(150)   # cover the ReduceScatter wait

                # --- S8: pad, outdconv 3x3, relu^2 ---
                yy = tp2.tile([128, CT * P], BF16, tag="yy")
                for ct in range(CT):
                    nc.gpsimd.dma_start(ocp4[:, ct, 2:34, 2:34],
                                        oc_bo[ct, :, :].rearrange("p (a b) -> p a b", a=HW))
                    for nh in range(2):
                        ps = pst.tile([128, 512], F32, tag="e")
                        for tap in range(9):
                            dy, dx = tap // 3, tap % 3
                            rhs = ocp4[:, ct, nh * 16 + 1 + dy:nh * 16 + 17 + dy,
                                       1 + dx:33 + dx]
                            nc.tensor.matmul(ps[:],
                                             dw3_sb[:, (ct * 9 + tap) * 128:(ct * 9 + tap + 1) * 128],
                                             rhs, start=(tap == 0), stop=(tap == 8))
                        y0 = tev.tile([128, 512], BF16, tag="y0")
                        nc.scalar.activation(y0[:], ps[:], AF.Relu)
                        nc.vector.tensor_mul(yy[:, ct * P + nh * 512:ct * P + (nh + 1) * 512],
                                             y0[:], y0[:])
                        if ct == 0 and nh == 0:
                            cpoke(y0[0:1, 0:16])

                # --- S9: outproj partial + one bf16 ReduceScatter over pixels ---
                op_bi = dramp.tile([P, D], BF16, tag="op_bi")
                op_bo = dramp.tile([P // 2, D], BF16, tag="op_bo")
                for mt in range(8):
                    ps = pst.tile([128, D], F32, tag="op")
                    for n in range(2):
                        nn = 512 if n == 0 else 256
                        for k in range(CT):
                            nc.tensor.matmul(ps[:, n * 512:n * 512 + nn],
                                             yy[:, k * P + mt * 128:k * P + (mt + 1) * 128],
                                             w_op_sb[:, k * D + n * 512:k * D + n * 512 + nn],
                                             start=(k == 0), stop=(k == CT - 1))
                    oevict = tev.tile([128, D], BF16, tag="oevict")
                    nc.scalar.activation(oevict[:], ps[:], AF.Copy)
                    nc.sync.dma_start(op_bi[mt * 128:(mt + 1) * 128, :], oevict[:])
                nc.gpsimd.collective_compute(
                    "ReduceScatter", OP.add, replica_groups=REPLICA_PAIRS,
                    ins=[op_bi.opt()], outs=[op_bo.opt()])
                # readback, cast to f32, write the output rows
                for chunk in range(4):
                    yf = tev.tile([128, D], BF16, tag="yf")
                    nc.sync.dma_start(yf[:], op_bo[chunk * 128:(chunk + 1) * 128, :])
                    yo = tev.tile([128, D], F32, tag="yo")
                    nc.vector.tensor_copy(yo[:], yf[:])
                    nc.sync.dma_start(y_out[chunk * 128:(chunk + 1) * 128, :], yo[:])

    nc.compile()
    return nc


# ======================= host side =======================

def _prep_weights(inputs):
    """Per-half host-folded weight tensors (numpy, bf16)."""
    bf = ml_dtypes.bfloat16
    norm_w = np.asarray(inputs["norm_w"], np.float64)
    norm_b = np.asarray(inputs["norm_b"], np.float64)
    in_proj_w = np.asarray(inputs["in_proj_w"], np.float64)
    conv7_w = np.asarray(inputs["conv7_w"], np.float64)
    conv7_b = np.asarray(inputs["conv7_b"], np.float64)
    xdown_w = np.asarray(inputs["xdown_w"], np.float64)
    wup_w = np.asarray(inputs["wup_w"], np.float64)
    lup_w = np.asarray(inputs["lup_w"], np.float64)
    uup_w = np.asarray(inputs["uup_w"], np.float64)
    dcoef_w = np.asarray(inputs["dcoef_w"], np.float64)
    m_w = np.asarray(inputs["m_w"], np.float64)
    outconv_w = np.asarray(inputs["outconv_w"], np.float64)
    outdconv_w = np.asarray(inputs["outdconv_w"], np.float64)
    outproj_w = np.asarray(inputs["outproj_w"], np.float64)

    Wf = in_proj_w * norm_w[None, :]
    b_in_full = in_proj_w @ norm_b

    idx = np.arange(128)
    out = []
    for half in range(2):
        rows = slice(half * DH, (half + 1) * DH)
        w_in = Wf[rows, :].T.reshape(6, 128, DH).transpose(1, 0, 2).reshape(128, 6 * DH)
        b_in = b_in_full[rows].reshape(1, DH)

        w7 = conv7_w[rows, 0].reshape(CT, 128, 49)           # (ct, k, tap)
        dw7 = np.zeros((CT, 128, 49, 128))
        for ct in range(CT):
            dw7[ct, idx, :, idx] = w7[ct]                    # (128, 49)
        dw7 = dw7.reshape(CT, 128, 49 * 128)
        b7h = conv7_b[rows].reshape(1, DH)

        w_xd = xdown_w[:, rows].T.reshape(CT, 128, 48).transpose(1, 0, 2).reshape(128, CT * 48)

        # gate weights: [Gl | Gm | Gr | L | U' | D'] each 12 mtiles (dir-major)
        blocks = []
        for ti in range(6):
            for c in range(12):
                k, c3 = c // 3, c % 3
                g0 = k * D + half * DH + c3 * 128
                if ti < 3:
                    src = wup_w[ti * 4 * D + g0: ti * 4 * D + g0 + 128]
                elif ti == 3:
                    src = lup_w[g0:g0 + 128]
                elif ti == 4:
                    src = uup_w[g0:g0 + 128] * m_w[k]
                else:
                    src = dcoef_w[g0:g0 + 128] * m_w[k]
                blocks.append(src)                            # (128, 48)
        wg = np.concatenate(blocks, axis=0).T                 # (48, 9216)
        w_gf = np.zeros((128, 72 * 128))
        w_gf[0:48] = wg
        w_gf[64:112] = wg

        w_oc = outconv_w[:, rows].T.reshape(CT, 128, D).transpose(1, 0, 2).reshape(128, CT * D)

        w3 = outdconv_w[rows, 0].reshape(CT, 128, 9)
        dw3 = np.zeros((128, CT, 9, 128))
        for ct in range(CT):
            dw3[idx, ct, :, idx] = w3[ct]
        dw3 = dw3.reshape(128, CT * 9 * 128)

        w_op = outproj_w[:, rows].T.reshape(CT, 128, D).transpose(1, 0, 2).reshape(128, CT * D)

        out.append(dict(
            w_in=w_in.astype(bf), b_in=b_in.astype(bf), dw7=dw7.astype(bf),
            b7=b7h.astype(bf), w_xd=w_xd.astype(bf), w_g=w_gf.astype(bf),
            w_oc=w_oc.astype(bf), dw3=dw3.astype(bf), w_op=w_op.astype(bf),
            ident=np.eye(128).astype(bf),
        ))
    return out


_CACHE = {}


def kernel(**inputs):
    if "nc" not in _CACHE:
        _CACHE["nc"] = build_program()
    nc = _CACHE["nc"]
    halves = _prep_weights(inputs)
    hs_full = np.asarray(inputs["hidden_states"], np.float32)
    in_maps = []
    for core in range(8):
        b, half = core // 2, core % 2
        m = dict(halves[half])
        m["hs"] = np.ascontiguousarray(hs_full[b])
        in_maps.append(m)
    res = bass_utils.run_bass_kernel_spmd(nc, in_maps, core_ids=list(range(8)))
    out = np.empty((4, P, D), np.float32)
    for core in range(8):
        b, half = core // 2, core % 2
        out[b, half * 512:(half + 1) * 512, :] = res.results[core]["y"]
    return out, hs_full


def _make_in_maps(inputs):
    halves = _prep_weights(inputs)
    hs_full = np.asarray(inputs["hidden_states"], np.float32)
    in_maps = []
    for core in range(8):
        b, half = core // 2, core % 2
        m = dict(halves[half])
        m["hs"] = np.ascontiguousarray(hs_full[b])
        in_maps.append(m)
    return in_maps


def bench(inputs, iters=20):
    """Device-resident repeated execution; returns median per-call wall ns."""
    import time
    import jax
    from jax.experimental.shard_map import shard_map
    from jax.sharding import Mesh, PartitionSpec
    from concourse import bass2jax, mybir as _mb

    if "nc" not in _CACHE:
        _CACHE["nc"] = build_program()
    nc = _CACHE["nc"]
    in_maps = _make_in_maps(inputs)
    bass2jax.install_neuronx_cc_hook()

    n_cores = 8
    in_names, out_names, out_avals, zero_outs = [], [], [], []
    partition_name = nc.partition_id_tensor.name if nc.partition_id_tensor else None
    for alloc in nc.m.functions[0].allocations:
        if not isinstance(alloc, _mb.MemoryLocationSet):
            continue
        name = alloc.memorylocations[0].name
        if alloc.kind == "ExternalInput":
            if name != partition_name:
                in_names.append(name)
        elif alloc.kind == "ExternalOutput":
            shape = tuple(alloc.tensor_shape)
            dtype = _mb.dt.np(alloc.dtype)
            out_names.append(name)
            out_avals.append(jax.core.ShapedArray(shape, dtype))
            zero_outs.append(np.zeros(shape, dtype))
    n_params = len(in_names)
    all_in_names = list(in_names) + list(out_names)
    if partition_name is not None:
        all_in_names.append(partition_name)

    import jax.numpy as jnp
    hs_idx = in_names.index("hs")

    def _make_body(reps):
        def _body(*args):
            operands = list(args)
            outs = None
            for _ in range(reps):
                ops = list(operands)
                if partition_name is not None:
                    ops.append(bass2jax.partition_id_tensor())
                outs = bass2jax._bass_exec_p.bind(
                    *ops, out_avals=tuple(out_avals), in_names=tuple(all_in_names),
                    out_names=tuple(out_names), lowering_input_output_aliases=(),
                    sim_require_finite=True, sim_require_nnan=True, nc=nc)
                y = outs[0]
                pad = jnp.concatenate([y, y], axis=0).astype(jnp.float32)
                operands[hs_idx] = operands[hs_idx] + 0.0 * pad
            return tuple(outs)
        return _body
    _body = _make_body(1)

    devices = jax.devices()[:n_cores]
    mesh = Mesh(np.asarray(devices), ("core",))
    nspec = (PartitionSpec("core"),) * (n_params + len(out_names))
    concat_in = [np.concatenate([np.asarray(in_maps[c][nm]) for c in range(n_cores)], axis=0)
                 for nm in in_names]
    concat_zero = [np.zeros((n_cores * z.shape[0], *z.shape[1:]), z.dtype) for z in zero_outs]
    sharding = jax.sharding.NamedSharding(mesh, PartitionSpec("core"))
    dev_args = [jax.device_put(a, sharding) for a in concat_in + concat_zero]

    def timed(reps, n):
        fn = jax.jit(shard_map(_make_body(reps), mesh=mesh, in_specs=nspec,
                               out_specs=(PartitionSpec("core"),) * len(out_names),
                               check_rep=False), keep_unused=True)
        r = fn(*dev_args)
        jax.block_until_ready(r)
        ts = []
        for _ in range(n):
            t0 = time.perf_counter()
            r = fn(*dev_args)
            jax.block_until_ready(r)
            ts.append(time.perf_counter() - t0)
        ts.sort()
        return ts[len(ts) // 2]

    # The axon tunnel adds a fixed ~70-85ms dispatch floor per jit call, which
    # swamps the actual kernel time.  Measure the true per-execution hardware
    # time as the marginal cost of extra chained executions inside one call:
    # each rep's input data-depends on the previous rep's output, so the
    # executions serialize on-device.
    t1 = timed(1, iters)
    for reps in (32, 16, 8):
        try:
            tn = timed(reps, max(4, iters // 4))
        except Exception as e:  # axon chain limits — fall back to fewer reps
            print(f"[bench] {reps}-rep chain failed ({type(e).__name__}); retrying")
            continue
        marginal = (tn - t1) / (reps - 1)
        print(f"[bench] per-call wall: {t1*1e3:.2f}ms (axon dispatch floor); "
              f"{reps}-rep chain: {tn*1e3:.2f}ms -> {marginal*1e6:.1f}us/exec")
        return marginal * 1e9
    print(f"[bench] chained measurement unavailable; reporting wall {t1*1e3:.2f}ms")
    return t1 * 1e9
